# revision 6
# baseline (speedup 1.0000x reference)
"""Trainium2 Bass kernel: Convpass adapter with hypernet-generated 3x3 conv.

Pipeline per core (data-parallel over batch, 8 images/core):
  hypernet: conv_w = emb @ w_hyper + b_hyper     (diag-window matmul trick)
  down:     x[pix,512] @ w_down -> [64, pix]     (PE-transpose + matmul)
  gelu1:    quickgelu(. + b_down)                (ACT sigmoid + DVE mul)
  conv:     3x3, stride 1, pad 1                 (9 shifted-window matmuls)
  gelu2:    quickgelu(. * scale)
  up:       y @ w_up + b_up -> [pix, 512]        (ones-row fused bias)

Matmul inputs are bf16 (cast during SWDGE DMA load); accumulation is fp32 in
PSUM and the final up-projection result is stored/written in fp32.
"""

import numpy as np

import concourse.bass as bass
import concourse.mybir as mybir
import concourse.tile as tile
from concourse import bacc
from concourse.bass_utils import run_bass_kernel_spmd
from concourse.masks import make_identity

# Problem shapes (hardcoded per contract).
B, H, W, C = 64, 28, 28, 512
DIM, EMB = 64, 64
NCORES = 8
B_LOC = B // NCORES            # 8 images per core
PIX = H * W                    # 784 pixels per image
PW = W + 2                     # 30 padded width
PAD = PW * (H + 2)             # 900 padded pixels per image
RH = 2                         # row-halves per image
RROWS = H // RH                # 14 rows per half
NHALF = RROWS * W              # 392 pixels per half-tile
PSUB = 112                     # pixel subtile (partition dim for transposes)
NSUB = PIX // PSUB             # 7 subtiles per image
KCH = C // 128                 # 4 contraction chunks of 128 channels
JTOT = DIM * DIM * 9           # 36864 hypernet outputs
OHALF = 32                     # o-channels per hypernet psum half
NHYP = OHALF * 9               # 288 = free size of hypernet matmuls

F32 = mybir.dt.float32
BF16 = mybir.dt.bfloat16
GELU_A = 1.702

_CACHE = {}


def build_kernel():
    if "nc" in _CACHE:
        return _CACHE["nc"]

    nc = bacc.Bacc("TRN2", target_bir_lowering=False, debug=False)

    x_d = nc.dram_tensor("x", [B_LOC * PIX, C], F32, kind="ExternalInput")
    wd_d = nc.dram_tensor("w_down", [C, DIM], F32, kind="ExternalInput")
    bd_d = nc.dram_tensor("b_down", [DIM], F32, kind="ExternalInput")
    wu_d = nc.dram_tensor("w_up", [DIM, C], F32, kind="ExternalInput")
    bu_d = nc.dram_tensor("b_up", [C], F32, kind="ExternalInput")
    sc_d = nc.dram_tensor("scale", [DIM], F32, kind="ExternalInput")
    emb_d = nc.dram_tensor("layer_emb", [EMB], F32, kind="ExternalInput")
    wh_d = nc.dram_tensor("w_hyper", [EMB, JTOT], F32, kind="ExternalInput")
    bh_d = nc.dram_tensor("b_hyper", [JTOT], F32, kind="ExternalInput")
    out_d = nc.dram_tensor("out", [B_LOC * PIX, C], F32, kind="ExternalOutput")

    with tile.TileContext(nc) as tc:
        with tc.tile_pool(name="consts", bufs=1) as consts:
            # ---- constants / small params ----
            ident = consts.tile([128, 128], BF16)
            make_identity(nc, ident[:])

            w_down_sb = consts.tile([128, KCH, DIM], BF16)
            nc.gpsimd.dma_start(
                w_down_sb[:], wd_d[:].rearrange("(k p) d -> p k d", p=128)
            )
            w_up65 = consts.tile([DIM + 1, C], BF16)
            nc.gpsimd.dma_start(w_up65[:DIM, :], wu_d[:])
            nc.gpsimd.dma_start(w_up65[DIM : DIM + 1, :], bu_d[:][None, :])

            b_down_sb = consts.tile([DIM, 1], F32)
            nc.sync.dma_start(b_down_sb[:], bd_d[:][:, None])
            b_down_g = consts.tile([DIM, 1], F32)
            nc.vector.tensor_scalar_mul(b_down_g[:], b_down_sb[:], GELU_A)

            scale_sb = consts.tile([DIM, 1], F32)
            nc.sync.dma_start(scale_sb[:], sc_d[:][:, None])
            scale_g = consts.tile([DIM, 1], F32)
            nc.vector.tensor_scalar_mul(scale_g[:], scale_sb[:], GELU_A)

            # ---- hypernet: W[i, o*9+t] = sum_e emb[e]*wh[e, o*576+i*9+t] + bh ----
            # T1: zeros except column 64 = emb; lhsT_i = T1[:, 64-i : 128-i]
            # places emb in output-partition i only.
            t1 = consts.tile([EMB, 128], BF16)
            nc.gpsimd.memset(t1[:], 0.0)
            nc.gpsimd.dma_start(t1[:, 64:65], emb_d[:][:, None])

            b_sb = consts.tile([DIM, DIM * 9], F32)
            with nc.allow_non_contiguous_dma(reason="tiny strided bias gather"):
                nc.sync.dma_start(
                    b_sb[:].rearrange("i (o t) -> i o t", t=9),
                    bh_d[:].rearrange("(o i t) -> i o t", o=DIM, i=DIM),
                )

            w_conv = consts.tile([DIM, DIM * 9], BF16)  # layout [i, o*9+t]

            with (
                tc.tile_pool(name="whpool", bufs=1) as whpool,
                tc.tile_pool(name="hpsum", bufs=1, space="PSUM") as hpsum,
            ):
                for h in range(2):
                    wh_sb = whpool.tile([EMB, JTOT // 2], BF16, tag="wh")
                    nc.gpsimd.dma_start(
                        wh_sb[:], wh_d[:, h * (JTOT // 2) : (h + 1) * (JTOT // 2)]
                    )
                    ps_w = hpsum.tile([DIM, 512], F32, tag="hp")
                    # [64e, 32o, 576j] view; per-i window = [o step 576][t step 1]
                    wh_v = wh_sb[:].rearrange("e (o j) -> e o j", j=DIM * 9)
                    for i in range(DIM):
                        nc.tensor.matmul(
                            ps_w[:, :NHYP],
                            t1[:, 64 - i : 128 - i],
                            wh_v[:, :, i * 9 : i * 9 + 9],
                            start=(i == 0),
                            stop=(i == DIM - 1),
                        )
                    nc.vector.tensor_tensor(
                        w_conv[:, h * NHYP : (h + 1) * NHYP],
                        ps_w[:, :NHYP],
                        b_sb[:, h * NHYP : (h + 1) * NHYP],
                        mybir.AluOpType.add,
                    )
            w_conv_v = w_conv[:].rearrange("i (o t) -> i o t", t=9)

            # ---- main per-image pipeline ----
            with (
                tc.tile_pool(name="xin", bufs=2) as xin,
                tc.tile_pool(name="xt", bufs=2) as xtp,
                tc.tile_pool(name="xact", bufs=3) as xactp,
                tc.tile_pool(name="yact", bufs=3) as yactp,
                tc.tile_pool(name="tmp", bufs=4) as tmpp,
                tc.tile_pool(name="outs", bufs=2) as outsp,
                tc.tile_pool(name="ps_t", bufs=2, space="PSUM") as ps_tp,
                tc.tile_pool(name="ps_s", bufs=4, space="PSUM") as ps_sp,
                tc.tile_pool(name="ps_u", bufs=2, space="PSUM") as ps_up,
            ):
                for img in range(B_LOC):
                    # 1) load x for this image (cast fp32 -> bf16): [112, 7, 512]
                    x_sb = xin.tile([PSUB, NSUB, C], BF16, tag="x")
                    nc.gpsimd.dma_start(
                        x_sb[:],
                        x_d[img * PIX : (img + 1) * PIX, :].rearrange(
                            "(s p) c -> p s c", p=PSUB
                        ),
                    )

                    # 2) transpose to xT [128ch, 4k, 784pix]
                    xT = xtp.tile([128, KCH, PIX], BF16, tag="xt")
                    for k in range(KCH):
                        for grp, (s0, ns) in enumerate(((0, 4), (4, 3))):
                            ps_x = ps_tp.tile([128, 448], BF16, tag="pst")
                            for s in range(s0, s0 + ns):
                                nc.tensor.transpose(
                                    ps_x[:, (s - s0) * PSUB : (s - s0 + 1) * PSUB],
                                    x_sb[:, s, k * 128 : (k + 1) * 128],
                                    ident[:PSUB, :PSUB],
                                )
                            cp = ps_x[:, : ns * PSUB]
                            dst = xT[:, k, s0 * PSUB : (s0 + ns) * PSUB]
                            if grp == 0:
                                nc.scalar.copy(dst, cp)
                            else:
                                nc.vector.tensor_copy(dst, cp)

                    # padded activation buffer [64, 30*30]
                    x_act = xactp.tile([DIM, PAD], BF16, tag="xa")
                    nc.gpsimd.memset(x_act[:], 0.0)
                    x_act_v = x_act[:].rearrange("d (r c) -> d r c", c=PW)

                    y_act = yactp.tile([DIM + 1, PIX], BF16, tag="ya")
                    nc.vector.memset(y_act[DIM : DIM + 1, :], 1.0)

                    for rh in range(RH):
                        # 3) down-proj -> psum [64, 392]
                        ps_d = ps_sp.tile([DIM, NHALF], F32, tag="pss")
                        for k in range(KCH):
                            nc.tensor.matmul(
                                ps_d[:],
                                w_down_sb[:, k, :],
                                xT[:, k, rh * NHALF : (rh + 1) * NHALF],
                                start=(k == 0),
                                stop=(k == KCH - 1),
                            )
                        # 4) quickgelu -> padded interior
                        t_t = tmpp.tile([DIM, NHALF], BF16, tag="t")
                        nc.vector.tensor_scalar_add(t_t[:], ps_d[:], b_down_sb[:])
                        s_t = tmpp.tile([DIM, NHALF], BF16, tag="s")
                        nc.scalar.activation(
                            s_t[:],
                            ps_d[:],
                            mybir.ActivationFunctionType.Sigmoid,
                            bias=b_down_g[:],
                            scale=GELU_A,
                        )
                        nc.vector.tensor_tensor(
                            x_act_v[:, 1 + rh * RROWS : 1 + (rh + 1) * RROWS, 1 : 1 + W],
                            t_t[:].rearrange("d (r c) -> d r c", c=W),
                            s_t[:].rearrange("d (r c) -> d r c", c=W),
                            mybir.AluOpType.mult,
                        )

                    for rh in range(RH):
                        # 5) conv: 9 shifted-window matmuls -> psum [64, 392]
                        ps_c = ps_sp.tile([DIM, NHALF], F32, tag="pss")
                        for t in range(9):
                            dy, dx = t // 3, t % 3
                            # strided window AP: [rows step PW][cols step 1]
                            src = x_act[:].rearrange("d (r c) -> d r c", c=PW)
                            src = src[
                                :,
                                rh * RROWS + dy : rh * RROWS + dy + RROWS,
                                dx : dx + W,
                            ]
                            nc.tensor.matmul(
                                ps_c[:],
                                w_conv_v[:, :, t],
                                src,
                                start=(t == 0),
                                stop=(t == 8),
                            )
                        # 6) quickgelu(scale * y)
                        t2 = tmpp.tile([DIM, NHALF], BF16, tag="t")
                        nc.vector.tensor_scalar_mul(t2[:], ps_c[:], scale_sb[:])
                        s2 = tmpp.tile([DIM, NHALF], BF16, tag="s")
                        nc.scalar.activation(
                            s2[:],
                            ps_c[:],
                            mybir.ActivationFunctionType.Sigmoid,
                            bias=0.0,
                            scale=scale_g[:],
                        )
                        nc.vector.tensor_tensor(
                            y_act[:DIM, rh * NHALF : (rh + 1) * NHALF],
                            t2[:],
                            s2[:],
                            mybir.AluOpType.mult,
                        )

                    # 7) up-proj + bias (ones row) -> [112, 7, 512] -> HBM
                    o_sb = outsp.tile([PSUB, NSUB, C], F32, tag="o")
                    for pt in range(NSUB):
                        ps_u = ps_up.tile([PSUB, C], F32, tag="psu")
                        nc.tensor.matmul(
                            ps_u[:],
                            y_act[:, pt * PSUB : (pt + 1) * PSUB],
                            w_up65[:],
                            start=True,
                            stop=True,
                        )
                        nc.scalar.copy(o_sb[:, pt, :], ps_u[:])
                    nc.sync.dma_start(
                        out_d[img * PIX : (img + 1) * PIX, :].rearrange(
                            "(s p) c -> p s c", p=PSUB
                        ),
                        o_sb[:],
                    )

    nc.compile()
    _CACHE["nc"] = nc
    return nc


def _make_in_maps(inputs):
    x = np.ascontiguousarray(inputs["x"], dtype=np.float32)
    shared = {
        k: np.ascontiguousarray(inputs[k], np.float32)
        for k in (
            "w_down",
            "b_down",
            "w_up",
            "b_up",
            "scale",
            "layer_emb",
            "w_hyper",
            "b_hyper",
        )
    }
    in_maps = []
    for c in range(NCORES):
        xc = x[c * B_LOC : (c + 1) * B_LOC].reshape(B_LOC * PIX, C)
        in_maps.append({"x": np.ascontiguousarray(xc), **shared})
    return in_maps


def kernel(**inputs) -> np.ndarray:
    nc = build_kernel()
    in_maps = _make_in_maps(inputs)
    res = run_bass_kernel_spmd(nc, in_maps, core_ids=list(range(NCORES)))
    outs = [res.results[c]["out"].reshape(B_LOC, H, W, C) for c in range(NCORES)]
    return np.concatenate(outs, axis=0)


def run_traced(inputs, **kw):
    """For test.py: run with tracing to get HW exec time."""
    nc = build_kernel()
    in_maps = _make_in_maps(inputs)
    return run_bass_kernel_spmd(
        nc, in_maps, core_ids=list(range(NCORES)), trace=True, **kw
    )
